# revision 77
# baseline (speedup 1.0000x reference)
"""Multi-head attention (B=4, S=2048, D=1024, H=16, causal) on 8 trn2 cores.

Sharding: data-parallel over batch (4) x tensor-parallel over head groups (2).
Core c handles batch b=c//2, heads g=c%2 (8 heads each). Each core computes
its partial output projection into fp16; host sums the two partials per batch
and adds the bias.

Single software-pipelined emission per core (fp16 matmuls, fp32 accum):
  - s-blocks processed in a small/big balanced pair order; kT projected
    up-front (dense prefix), qT per column-slab just in time, v spread via a
    background job queue; X^T slabs page through small rotating pools.
  - per unit (s-block i, head pair m, half z): scores [128, W] in PSUM
    512-chunks; the score matmul opens each chunk's accumulation group and
    a narrow N=128 ident.T @ mask matmul adds the causal mask onto the
    diagonal 128 columns and closes the group (keeps the mask off the PE
    critical path),
    per-chunk row-max + negated combine (DVE), one exp per chunk on ACT
    (bias=-rowmax) -> E fp16, E -> E^T via DMA-transpose XBAR (SP queue),
    PV with lhsT=E^T tile, rhs=[v_tile | 1] -> out[s, dk+1]; the 65th
    column accumulates the softmax denominator via the ones column.
  - concat[s, hd] = PV * (1/den) (DVE recip; scales split DVE / ACT),
    DMA-transpose -> concat^T, output projection (K=512) -> PSUM -> fp16 ->
    DRAM; x-slab loads and y stores ride the Pool SWDGE queue.
"""

import math

import numpy as np

B, S, D, H = 4, 2048, 1024, 16
DK = 64
HLOC = 8          # heads per core
HD = HLOC * DK    # 512 local concat dims
P = 128
NBLK = S // P     # 16 s-blocks
KO = D // P       # 8 contraction tiles for projections
MPAIRS = 4        # head pairs per core
NEG = 30000.0     # mask addend on negated scores
CH = 512          # score psum chunk width (1 bank)
LAG = 8           # units between scores emission and PV emission
LOOKAHEAD = 5

# big/small pairs so the DVE/ACT-heavy big blocks interleave with cheap ones
BLOCKS = [15, 0, 14, 1, 13, 2, 12, 3, 11, 4, 10, 5, 9, 6, 8, 7]


def build():
    import concourse.bass as bass  # noqa: F401
    import concourse.mybir as mybir
    import concourse.tile as tile
    from concourse import bacc

    fp16 = mybir.dt.float16
    f32 = mybir.dt.float32
    Alu = mybir.AluOpType
    Ax = mybir.AxisListType

    nc = bacc.Bacc()

    xtq = nc.dram_tensor("xtq", [D, S], fp16, kind="ExternalInput")
    xtk = nc.dram_tensor("xtk", [D, S], fp16, kind="ExternalInput")
    xtv = nc.dram_tensor("xtv", [D, S], fp16, kind="ExternalInput")
    wq = nc.dram_tensor("wq", [D, HD], fp16, kind="ExternalInput")
    wk = nc.dram_tensor("wk", [D, HD], fp16, kind="ExternalInput")
    wv = nc.dram_tensor("wv", [D, HD], fp16, kind="ExternalInput")
    wo = nc.dram_tensor("wo", [HD, D], fp16, kind="ExternalInput")
    masktri = nc.dram_tensor("masktri", [P, CH], fp16, kind="ExternalInput")
    ident = nc.dram_tensor("ident", [P, P], fp16, kind="ExternalInput")
    y = nc.dram_tensor("y", [S, D], fp16, kind="ExternalOutput")

    with tile.TileContext(nc) as tc:
        with (
            tc.tile_pool(name="persist", bufs=1) as persist,
            tc.tile_pool(name="stats", bufs=32) as stats,
            tc.tile_pool(name="xq", bufs=2) as xqpool,
            tc.tile_pool(name="xv", bufs=2) as xvpool,
            tc.tile_pool(name="chunks", bufs=6, space="PSUM") as chunkpool,
            tc.tile_pool(name="pvp", bufs=1, space="PSUM") as pvpool,
            tc.tile_pool(name="csb", bufs=2) as cpool,
            tc.tile_pool(name="ct", bufs=2) as ctpool,
            tc.tile_pool(name="ysb", bufs=2) as ysbpool,
        ):
            wq_sb = persist.tile([P, KO, HD], fp16, tag="wq", name="wq")
            wk_sb = persist.tile([P, KO, HD], fp16, tag="wk", name="wk")
            wv_sb = persist.tile([P, KO, HD], fp16, tag="wv", name="wv")
            nc.sync.dma_start(out=wk_sb, in_=wk[:].rearrange("(ko p) n -> p ko n", p=P))
            nc.sync.dma_start(out=wq_sb, in_=wq[:].rearrange("(ko p) n -> p ko n", p=P))
            mtri_sb = persist.tile([P, CH], fp16, tag="mtri", name="mtri")
            nc.sync.dma_start(out=mtri_sb, in_=masktri[:])
            ident_sb = persist.tile([P, P], fp16, tag="ident", name="ident")
            nc.sync.dma_start(out=ident_sb, in_=ident[:])
            nc.sync.dma_start(out=wv_sb, in_=wv[:].rearrange("(ko p) n -> p ko n", p=P))
            wo_sb = persist.tile([P, MPAIRS, D], fp16, tag="wo", name="wo")
            nc.sync.dma_start(out=wo_sb, in_=wo[:].rearrange("(m p) n -> p m n", p=P))

            qt = persist.tile([P, MPAIRS, S], fp16, tag="qt", name="qt")  # hd%128
            kt = persist.tile([P, MPAIRS, S], fp16, tag="kt", name="kt")
            # v + ones column: [t%128, t//128, h, dk|1]
            vv = persist.tile([P, NBLK, HLOC, DK + 1], fp16, tag="vv", name="vv")
            nc.gpsimd.memset(vv[:, :, :, DK : DK + 1], 1.0)

            xq_r = xtq[:].rearrange("(ko p) s -> p ko s", p=P)
            xk_r = xtk[:].rearrange("(ko p) s -> p ko s", p=P)
            xv_r = xtv[:].rearrange("(ko p) s -> p ko s", p=P)

            units = [(i, m, z) for i in BLOCKS for m in range(MPAIRS)
                     for z in (0, 1)]
            NU = len(units)  # 128

            emitted = set()
            slabs = {}

            def ensure_slab(t, s):
                key = ("slab", t, s)
                if key in emitted:
                    return
                emitted.add(key)
                pool, src = {"q": (xqpool, xq_r), "k": (xkpools[0], xk_r),
                             "v": (xvpool, xv_r)}[t]
                xsb = pool.tile([P, KO, 512], fp16, tag="x" + t, name="xs")
                nc.gpsimd.dma_start(out=xsb, in_=src[:, :, s * 512 : (s + 1) * 512])
                slabs[key] = xsb

            def ensure_kt(m, c):
                key = ("kt", m, c)
                if key in emitted:
                    return
                emitted.add(key)
                ensure_slab("k", c)
                xsb = slabs[("slab", "k", c)]
                ps = chunkpool.tile([P, CH], f32, tag="ck", name="ck")
                for ko in range(KO):
                    nc.tensor.matmul(
                        ps[:, 0:512],
                        lhsT=wk_sb[:, ko, m * P : (m + 1) * P],
                        rhs=xsb[:, ko, :],
                        start=(ko == 0),
                        stop=(ko == KO - 1),
                    )
                nc.scalar.copy(
                    out=kt[:, m, c * 512 : (c + 1) * 512], in_=ps[:, 0:512]
                )

            def ensure_qt(i, m):
                s = i // 4
                key = ("qt", s, m)
                if key in emitted:
                    return
                emitted.add(key)
                ensure_slab("q", s)
                xsb = slabs[("slab", "q", s)]
                ps = chunkpool.tile([P, CH], f32, tag="ck", name="ck")
                for ko in range(KO):
                    nc.tensor.matmul(
                        ps[:, 0:512],
                        lhsT=wq_sb[:, ko, m * P : (m + 1) * P],
                        rhs=xsb[:, ko, :],
                        start=(ko == 0),
                        stop=(ko == KO - 1),
                    )
                nc.scalar.copy(
                    out=qt[:, m, s * 512 : (s + 1) * 512], in_=ps[:, 0:512]
                )

            def ensure_v(tm):
                key = ("v", tm)
                if key in emitted:
                    return
                emitted.add(key)
                ensure_slab("v", tm // 4)
                xsb = slabs[("slab", "v", tm // 4)]
                co = (tm % 4) * P
                ps = chunkpool.tile([P, CH], f32, tag="ck", name="ck")
                for ko in range(KO):
                    nc.tensor.matmul(
                        ps[:, 0:512],
                        lhsT=xsb[:, ko, co : co + P],
                        rhs=wv_sb[:, ko, :],
                        start=(ko == 0),
                        stop=(ko == KO - 1),
                    )
                nc.vector.tensor_copy(
                    out=vv[:, tm, :, 0:DK],
                    in_=ps[:, 0:512].rearrange("p (h k) -> p h k", h=HLOC),
                )

            # background jobs (v projections then remaining qt blocks),
            # popped a few per unit step
            bg = [("v", tm) for tm in range(NBLK)]
            seen = set()
            for i in BLOCKS[2:]:
                if i // 4 not in seen:
                    seen.add(i // 4)
                    for m in range(MPAIRS):
                        bg.append(("qt", i, m))

            def pop_bg(n):
                done = 0
                while bg and done < n:
                    kind = bg.pop(0)
                    if kind[0] == "v":
                        if ("v", kind[1]) not in emitted:
                            ensure_v(kind[1])
                            done += 1
                    else:
                        if ("qt", kind[1], kind[2]) not in emitted:
                            ensure_qt(kind[1], kind[2])
                            done += 1

            state = {}
            pv_tiles = {}
            csb_tiles = {}
            ct_tiles = {}
            rden_tiles = {}
            pending = {}

            def emit_unit(u):
                i, m, z = units[u]
                W = (i + 1) * P
                off = z * DK
                nch = (W + CH - 1) // CH
                nmx = stats.tile([P, 4], f32, tag="nmx", name="nmx")
                chunk_tiles = []
                for cc in range(nch):
                    cw = min(CH, W - cc * CH)
                    c0 = cc * CH
                    ck = chunkpool.tile([P, CH], f32, tag="ck", name="ck")
                    chunk_tiles.append((ck, cw))
                    has_diag = c0 + cw == W
                    nc.tensor.matmul(
                        ck[:, 0:cw],
                        lhsT=qt[off : off + DK, m, i * P : (i + 1) * P],
                        rhs=kt[off : off + DK, m, c0 : c0 + cw],
                        start=True,
                        stop=not has_diag,
                    )
                    if has_diag:
                        nc.tensor.matmul(
                            ck[:, cw - P : cw],
                            lhsT=ident_sb,
                            rhs=mtri_sb[:, CH - P : CH],
                            start=False,
                            stop=True,
                        )
                    nc.vector.tensor_reduce(
                        nmx[:, cc : cc + 1], ck[:, 0:cw], axis=Ax.X, op=Alu.max,
                        negate=(nch == 1),
                    )
                if nch > 1:
                    negmx = stats.tile([P, 1], f32, tag="negmx", name="negmx")
                    nc.vector.tensor_reduce(
                        negmx, nmx[:, 0:nch], axis=Ax.X, op=Alu.max, negate=True
                    )
                else:
                    negmx = nmx[:, 0:1]
                if i >= 8:
                    ebuf = ebufpool.tile([P, S], fp16, tag="ebuf", name="ebuf")
                else:
                    ebuf = ebufspool.tile([P, S // 2], fp16, tag="ebuf", name="ebuf")
                for cc, (ck, cw) in enumerate(chunk_tiles):
                    nc.scalar.activation(
                        out=ebuf[:, cc * CH : cc * CH + cw],
                        in_=ck[:, 0:cw],
                        func=mybir.ActivationFunctionType.Exp,
                        bias=negmx,
                        scale=1.0,
                    )
                pool = ptsbig if i >= 8 else ptssmall
                nb = NBLK if i >= 8 else 8
                pts = pool.tile([P, nb, P], fp16, tag="pts", name="pts")
                nc.sync.dma_start(
                    out=pts[:, 0 : i + 1, :], in_=ebuf[:, 0:W], transpose=True
                )
                state[u] = pts

            def emit_pv(u):
                i, m, z = units[u]
                for tm in range(i + 1):
                    ensure_v(tm)
                pts = state.pop(u)
                if m == 0 and z == 0:
                    pv_tiles[i] = [
                        pvpool.tile([P, MPAIRS, DK + 1], f32, tag=f"pv{zz}",
                                    name=f"pv{zz}")
                        for zz in (0, 1)
                    ]
                pv = pv_tiles[i][z]
                for j in range(i + 1):
                    nc.tensor.matmul(
                        pv[:, m, :],
                        lhsT=pts[:, j, :],
                        rhs=vv[:, j, 2 * m + z, :],
                        start=(j == 0),
                        stop=(j == i),
                    )

            def emit_recip(i, m, z):
                if m == 0 and z == 0:
                    csb_tiles[i] = cpool.tile(
                        [P, HLOC, DK], fp16, tag="csb", name="csb"
                    )
                pv = pv_tiles[i][z]
                rden = stats.tile([P, 1], f32, tag="rden", name="rden")
                rden_tiles[(i, m, z)] = rden
                nc.vector.reciprocal(rden, pv[:, m, DK : DK + 1])

            def emit_scale(i, m, z):
                csb = csb_tiles[i]
                pv = pv_tiles[i][z]
                rden = rden_tiles.pop((i, m, z))
                if z == 0:
                    nc.vector.tensor_scalar_mul(
                        csb[:, 2 * m + z, :], pv[:, m, 0:DK], rden
                    )
                else:
                    nc.scalar.activation(
                        out=csb[:, 2 * m + z, :], in_=pv[:, m, 0:DK],
                        func=mybir.ActivationFunctionType.Copy, scale=rden,
                    )

            def emit_ct(i):
                csb = csb_tiles.pop(i)
                ct = ctpool.tile([P, MPAIRS, P], fp16, tag="ct", name="ct")
                ct_tiles[i] = ct
                nc.sync.dma_start(
                    out=ct, in_=csb[:].rearrange("p h k -> p (h k)"),
                    transpose=True,
                )

            def emit_outproj(i):
                ct = ct_tiles.pop(i)
                ysb = ysbpool.tile([P, D], fp16, tag="ysb", name="ysb")
                yps = chunkpool.tile([P, CH], f32, tag="ck", name="ck")
                for nch in range(2):
                    for kk in range(MPAIRS):
                        nc.tensor.matmul(
                            yps,
                            lhsT=ct[:, kk, :],
                            rhs=wo_sb[:, kk, nch * 512 : (nch + 1) * 512],
                            start=(kk == 0),
                            stop=(kk == MPAIRS - 1),
                        )
                    if nch == 0:
                        nc.vector.tensor_copy(
                            out=ysb[:, 0:512], in_=yps
                        )
                    else:
                        nc.scalar.copy(
                            out=ysb[:, 512:1024], in_=yps
                        )
                nc.gpsimd.dma_start(out=y[:][i * P : (i + 1) * P, :], in_=ysb)

            # ---- dense projection prefix: all of kT (slab pool scoped so
            # its SBUF is reused by the later pools), then qT for the first
            # two blocks in the schedule ----
            xk_ctx = tc.tile_pool(name="xk", bufs=2)
            xkpools = [xk_ctx.__enter__()]
            for c in range(4):
                for m in range(MPAIRS):
                    ensure_kt(m, c)
            xk_ctx.__exit__(None, None, None)
            ebuf_ctx = tc.tile_pool(name="ebuf", bufs=6)
            ebufpool = ebuf_ctx.__enter__()
            ebufs_ctx = tc.tile_pool(name="ebufs", bufs=6)
            ebufspool = ebufs_ctx.__enter__()
            ptsb_ctx = tc.tile_pool(name="ptsb", bufs=7)
            ptsbig = ptsb_ctx.__enter__()
            ptss_ctx = tc.tile_pool(name="ptss", bufs=6)
            ptssmall = ptss_ctx.__enter__()
            for m in range(MPAIRS):
                ensure_qt(BLOCKS[0], m)

            # ---- main interleaved emission loop ----
            for u in range(NU + LAG + 8):
                for fn in pending.pop(u, ()):
                    fn()
                if u < NU:
                    pop_bg(2 if u < 24 else 1)
                    for uu in range(u, min(u + 1 + LOOKAHEAD, NU)):
                        ensure_qt(units[uu][0], units[uu][1])
                    emit_unit(u)
                w = u - LAG
                if 0 <= w < NU:
                    emit_pv(w)
                    i, m, z = units[w]
                    emit_recip(i, m, z)
                    emit_scale(i, m, z)
                    if m == MPAIRS - 1 and z == 1:
                        pending.setdefault(u + 3, []).append(
                            lambda i=i: emit_ct(i)
                        )
                        pending.setdefault(u + 5, []).append(
                            lambda i=i: emit_outproj(i)
                        )
                if u >= NU:
                    pop_bg(2)
            for fns in [pending[k] for k in sorted(pending)]:
                for fn in fns:
                    fn()
            for ctx in (ptss_ctx, ptsb_ctx, ebufs_ctx, ebuf_ctx):
                ctx.__exit__(None, None, None)

    nc.finalize()
    return nc


def _prep_inputs(Q, K, V, Wq, Wk, Wv, Wo):
    """Host-side shard + layout prep. Returns list of 8 in_maps."""
    rt8 = math.sqrt(math.sqrt(64.0))  # sqrt(8): scale split over q and k
    in_maps = []
    # sliding diagonal-block causal mask (-NEG above the diagonal in the
    # last 128 columns) and an identity, fed to the PE so the mask lands in
    # PSUM via a matmul that opens each diagonal chunk's accumulation group
    tri = np.zeros((P, CH), np.float16)
    tri[:, CH - P :] = np.where(
        np.arange(P)[:, None] < np.arange(P)[None, :], np.float16(-NEG), 0.0
    )
    identity = np.eye(P, dtype=np.float16)
    for c in range(8):
        b, g = c // 2, c % 2
        heads = slice(g * HLOC, (g + 1) * HLOC)
        # [H,D,DK] -> [D, HLOC*DK]; q negated so row-max becomes row-min
        wq_p = (Wq[heads] * rt8).transpose(1, 0, 2).reshape(D, HD)
        wk_p = (Wk[heads] * rt8).transpose(1, 0, 2).reshape(D, HD)
        wv_p = Wv[heads].transpose(1, 0, 2).reshape(D, HD)
        wo_p = Wo[:, g * HD : (g + 1) * HD].T  # [HD, D]
        in_maps.append({
            "xtq": np.ascontiguousarray(Q[b].T).astype(np.float16),
            "xtk": np.ascontiguousarray(K[b].T).astype(np.float16),
            "xtv": np.ascontiguousarray(V[b].T).astype(np.float16),
            "wq": np.ascontiguousarray(wq_p).astype(np.float16),
            "wk": np.ascontiguousarray(wk_p).astype(np.float16),
            "wv": np.ascontiguousarray(wv_p).astype(np.float16),
            "wo": np.ascontiguousarray(wo_p).astype(np.float16),
            "masktri": tri,
            "ident": identity,
        })
    return in_maps


_NC = []


def kernel(Q, K, V, mask, Wq, Wk, Wv, Wo, bo, _trace=False):
    from concourse.bass_utils import run_bass_kernel_spmd

    Q, K, V = np.asarray(Q), np.asarray(K), np.asarray(V)
    Wq, Wk, Wv = np.asarray(Wq), np.asarray(Wk), np.asarray(Wv)
    Wo, bo = np.asarray(Wo), np.asarray(bo)

    if not _NC:
        _NC.append(build())
    nc = _NC[0]
    in_maps = _prep_inputs(Q, K, V, Wq, Wk, Wv, Wo)
    res = run_bass_kernel_spmd(nc, in_maps, core_ids=list(range(8)), trace=_trace)
    ys = [r["y"].astype(np.float32) for r in res.results]
    out = np.stack([ys[2 * b] + ys[2 * b + 1] for b in range(B)])
    out = out + bo[None, None, :].astype(np.float32)
    if _trace:
        kernel._last = res
    return out.astype(np.float32)

